# revision 41
# baseline (speedup 1.0000x reference)
"""Trainium2 Bass kernel for nn_MinBlcokScan: 4 grouped 1-D cross-correlations.

Math (reference): x = batch_x.reshape(B, 32, L). For each group g of 4,
channels [8g..8g+7] are convolved ('same', zero pad 2/2) with kernels_g
[4, 8, 5] producing out[:, 4g+o, :]; outputs concatenated to [B, 16*L].

Strategy: pure data parallel over batch (4 samples per core) plus a
polyphase-16 reformulation in bf16. For each (group g, sample s) = u, the
host packs x into a [128, L/16 + 2] tile whose partitions are (phase, chan).
Per 512-column PSUM block and sample-pair, six independent 128-row matmuls
(each start=stop=True — no in-bank accumulation chains, which this runtime
serializes and miscompiles at mixed row offsets) compute:

  bank A: main taps (in-column phases)   -> [0:64] sample0, [64:128] sample1
  bank D: cross-column taps, using 32-wide weight slabs whose only nonzero
          columns are the r{0,1} (left, x view shifted -1) and r{14,15}
          (right, +1) output phases -> 32-aligned regions mirroring bank A

The combine (ot = A + D, cast to bf16) runs per double-width bank pair:
an Act-engine PSUM->SBUF copy of D (TensorTensor reads at most one PSUM
operand on this ISA) then a Vector add. All DMA moves 128-partition tiles
with >=4KB contiguous descriptors. I/O is bf16 (fp32 PSUM accumulation),
rel err ~3e-3 vs the fp32 reference; the run is DMA-bound at ~25 MB/core.
"""

import numpy as np
from contextlib import ExitStack

import ml_dtypes

import concourse.bass as bass
import concourse.bacc as bacc
import concourse.mybir as mybir
import concourse.tile as tile
from concourse.bass_utils import run_bass_kernel_spmd

D = 32           # input channels
L = 65536        # sequence length
W = 5            # conv window
B = 32           # batch
N_CORES = 8
S = 4            # samples per core
M = L // 16      # polyphase columns per (g, s) = 4096
J = M + 2        # zero-padded columns in the x tile
NSUB = 512       # matmul moving free dim == one fp32 PSUM bank
NQ = M // NSUB   # 8 column blocks per u

F32 = mybir.dt.float32
BF16 = mybir.dt.bfloat16
NP_BF16 = ml_dtypes.bfloat16

# Pairs 8-K_AUX..7 take the aux-edge path: a host-baked [128, M] tile per
# TWO pairs replaces their per-block L/R matmuls (4) with one block-zero
# weighted 128-row matmul each — trades ~5 us PE per pair for ~1.5 us DMA.
# Must be even (aux tiles are shared by pair duos). Aux pairs sit at the
# END of the schedule with their tiles prefetched from the start, so the
# extra loads hide under earlier compute.
K_AUX = 4

# phase permutation kept from the earlier layout (any layout works for this
# design; PERM[0..1]=0..1 etc. retained for marshalling stability)
PERM = {0: 0, 1: 1, 2: 2, 3: 3, 14: 4, 15: 5}
for _ph in range(4, 14):
    PERM[_ph] = _ph + 2
INVPERM = [0] * 16
for _ph, _p in PERM.items():
    INVPERM[_p] = _ph


def build_program(k_aux=K_AUX):
    assert k_aux % 2 == 0
    nc = bacc.Bacc(trn_type="TRN2", target_bir_lowering=False, debug=False)
    x = nc.dram_tensor("x", [16 * 128, J], BF16, kind="ExternalInput").ap()
    xea = nc.dram_tensor("xea", [4 * 128, M], BF16, kind="ExternalInput").ap()
    wm = nc.dram_tensor("wm", [128, 256], BF16, kind="ExternalInput").ap()
    wl = nc.dram_tensor("wl", [128, 128], BF16, kind="ExternalInput").ap()
    wr = nc.dram_tensor("wr", [128, 128], BF16, kind="ExternalInput").ap()
    wea = nc.dram_tensor("wea", [128, 1024], BF16, kind="ExternalInput").ap()
    y = nc.dram_tensor("y", [16 * 64, M], BF16, kind="ExternalOutput").ap()

    with tile.TileContext(nc) as tc, ExitStack() as ctx:
        wp = ctx.enter_context(tc.tile_pool(name="wp", bufs=1))
        xp = ctx.enter_context(tc.tile_pool(name="xp", bufs=5))
        ep = ctx.enter_context(tc.tile_pool(name="ep", bufs=2))
        op = ctx.enter_context(tc.tile_pool(name="op", bufs=4))
        dp = ctx.enter_context(tc.tile_pool(name="dp", bufs=4))
        pp = ctx.enter_context(tc.tile_pool(name="pp", bufs=2, space="PSUM"))

        # weight loads ride the (otherwise idle at start) Act queue so their
        # issue overhead runs in parallel with the first x loads on SP
        wmt = wp.tile([128, 256], BF16)
        nc.scalar.dma_start(wmt[:], wm)
        wlt = wp.tile([128, 128], BF16)
        nc.scalar.dma_start(wlt[:], wl)
        wrt = wp.tile([128, 128], BF16)
        nc.scalar.dma_start(wrt[:], wr)
        weat = None
        if k_aux > 0:
            weat = wp.tile([128, 1024], BF16)
            nc.scalar.dma_start(weat[:], wea)

        eats = {}
        JH = J // 2
        for pr in range(8):          # sample-pair u = 2*pr, 2*pr+1 (same g)
            g = (2 * pr) // 4
            t, p = pr // 2, pr % 2
            aux = pr >= 8 - k_aux
            xts = []
            for h in range(2):
                u = 2 * pr + h
                xt = xp.tile([128, J], BF16, name=f"xt{h}")
                # column-split loads: the first blocks' matmuls unblock after
                # a fraction of the tile has landed (subtile deps track
                # column ranges); quarters for pair 0 shorten the cold start
                nsp = 4 if pr == 0 else 2
                step = (J + nsp - 1) // nsp
                for c in range(0, J, step):
                    ce = min(c + step, J)
                    nc.sync.dma_start(xt[:, c:ce], x[u * 128:(u + 1) * 128, c:ce])
                xts.append(xt)
            if pr == 2 and k_aux > 0:
                # prefetch trailing pairs' aux tiles now: far enough ahead to
                # hide, late enough not to delay pair 1's x loads
                for t2 in range((8 - k_aux) // 2, 4):
                    eat2 = ep.tile([128, M], BF16, name=f"eat{t2 % 2}")
                    nc.gpsimd.dma_start(eat2[:], xea[t2 * 128:(t2 + 1) * 128, :])
                    eats[t2] = eat2
            eat = eats.get(t)
            ot = op.tile([128, M], BF16)

            for qq in range(NQ // 2):   # two PSUM banks per tile: combine
                ptA = pp.tile([128, 2 * NSUB], F32, name="ptA")
                ptD = pp.tile([128, 2 * NSUB], F32, name="ptD")
                for qh in range(2):
                    q = qq * 2 + qh
                    c0 = q * NSUB
                    b0 = qh * NSUB
                    for h in range(2):
                        xt = xts[h]
                        # main taps
                        nc.tensor.matmul(
                            ptA[h * 64:(h + 1) * 64, b0:b0 + NSUB],
                            wmt[:, g * 64:(g + 1) * 64],
                            xt[:, 1 + c0:1 + c0 + NSUB],
                            start=True, stop=True, skip_group_check=True,
                        )
                        if not aux:
                            # left edge (x shifted -1): nonzero outs r{0,1}
                            nc.tensor.matmul(
                                ptD[h * 64:h * 64 + 32, b0:b0 + NSUB],
                                wlt[:, g * 32:(g + 1) * 32],
                                xt[:, c0:c0 + NSUB],
                                start=True, stop=True, skip_group_check=True,
                                tile_position=(0, h * 64),
                            )
                            # right edge (+1): nonzero outs r{14,15}
                            nc.tensor.matmul(
                                ptD[h * 64 + 32:(h + 1) * 64, b0:b0 + NSUB],
                                wrt[:, g * 32:(g + 1) * 32],
                                xt[:, 2 + c0:2 + c0 + NSUB],
                                start=True, stop=True, skip_group_check=True,
                                tile_position=(0, h * 64 + 32),
                            )
                    if aux:
                        # both samples' L+R edges in one 128-row matmul;
                        # rows of the duo's other pair carry zero weights
                        nc.tensor.matmul(
                            ptD[:, b0:b0 + NSUB],
                            weat[:, t * 256 + p * 128:t * 256 + (p + 1) * 128],
                            eat[:, c0:c0 + NSUB],
                            start=True, stop=True, skip_group_check=True,
                        )
                # TensorTensor may read at most one PSUM operand: stage D in
                # SBUF via the Act engine, then add A (PSUM) + D (SBUF).
                # Double-width ops amortize the per-op access bubble.
                cc = qq * 2 * NSUB
                ds = dp.tile([128, 2 * NSUB], F32, name="ds")
                nc.scalar.copy(ds[:], ptD[:])
                nc.vector.tensor_tensor(ot[:, cc:cc + 2 * NSUB], ptA[:], ds[:],
                                        mybir.AluOpType.add)

            if pr == 7:
                # split the final store: its first half only waits on the
                # first half's combines, trimming the drain tail
                MH = M // 2
                nc.gpsimd.dma_start(y[pr * 128:(pr + 1) * 128, 0:MH], ot[:, 0:MH])
                nc.gpsimd.dma_start(y[pr * 128:(pr + 1) * 128, MH:M], ot[:, MH:M])
            else:
                nc.gpsimd.dma_start(y[pr * 128:(pr + 1) * 128, :], ot[:])
    nc.compile()
    return nc


def build_weights(kernels):
    """wm [128, 256], wl [128, 128], wr [128, 128] (bf16).

    wm[PERM[ph]*8+c, g*64 + r*4+o] = k[o,c,t], ph = r+t-2 in [0,16).
    wl: out col g*32 + r*4+o (r in {0,1}), taps from ph' = 14+e of col m-1:
        wl[PERM[14+e]*8+c, g*32 + r*4+o] = k[o,c,e-r] for 0 <= e-r < W.
    wr: out col g*32 + 24 + (r-14)*4+o (r in {14,15}), ph' = e of col m+1:
        wr[PERM[e]*8+c, ...] = k[o,c,18+e-r] for valid t.
    """
    wm = np.zeros((128, 256), np.float32)
    wl = np.zeros((128, 128), np.float32)
    wr = np.zeros((128, 128), np.float32)
    for g, ker in enumerate(kernels):       # ker [4, 8, 5]
        for o in range(4):
            for c in range(8):
                for r in range(16):
                    for t in range(W):
                        ph = r + t - 2
                        if 0 <= ph < 16:
                            wm[PERM[ph] * 8 + c, g * 64 + r * 4 + o] = ker[o, c, t]
                for e in range(2):
                    for r in (0, 1):
                        t = e - r
                        if 0 <= t < W:
                            wl[PERM[14 + e] * 8 + c, g * 32 + r * 4 + o] = ker[o, c, t]
                    for r in (14, 15):
                        t = 18 + e - r
                        if 0 <= t < W:
                            wr[PERM[e] * 8 + c, g * 32 + 24 + (r - 14) * 4 + o] = ker[o, c, t]
    return (wm.astype(NP_BF16), wl.astype(NP_BF16), wr.astype(NP_BF16))


def build_wea(kernels):
    """wea [128, 1024] bf16: aux-edge weights for pair duo t (g = t).
    Col t*256 + p*128 + h*64 + r*4 + o; row p*64 + h*32 + side*16 + e*8 + c
    (rows of the duo's other parity are zero)."""
    wea = np.zeros((128, 1024), np.float32)
    for t in range(4):
        ker = kernels[t]
        for p in range(2):
            for h in range(2):
                for o in range(4):
                    for c in range(8):
                        for e in range(2):
                            for r in (0, 1):        # left: t_tap = e - r
                                tt = e - r
                                if 0 <= tt < W:
                                    wea[p * 64 + h * 32 + e * 8 + c,
                                        t * 256 + p * 128 + h * 64 + r * 4 + o] = ker[o, c, tt]
                            for r in (14, 15):      # right: t_tap = 18+e-r
                                tt = 18 + e - r
                                if 0 <= tt < W:
                                    wea[p * 64 + h * 32 + 16 + e * 8 + c,
                                        t * 256 + p * 128 + h * 64 + r * 4 + o] = ker[o, c, tt]
    return wea.astype(NP_BF16)


def marshal_xea(x4):
    """Aux-edge tiles [4*128, M] bf16: group t serves pairs (2t, 2t+1);
    row p*64 + h*32 + side*16 + e*8 + c; side 0 = position 16m-2+e (left),
    side 1 = 16m+16+e (right), zero-padded at the boundaries."""
    xea = np.zeros((4, 128, M), np.float32)
    idx = 16 * np.arange(M)
    for t in range(4):
        for p in range(2):
            pr = 2 * t + p
            for h in range(2):
                u = 2 * pr + h
                g, s = u // 4, u % 4
                ch = slice(8 * g, 8 * g + 8)
                for e in range(2):
                    pos = idx - 2 + e
                    ok = pos >= 0
                    r0 = p * 64 + h * 32 + e * 8
                    xea[t, r0:r0 + 8, ok] = x4[s, ch, pos[ok]]
                    pos = idx + 16 + e
                    ok = pos < L
                    r0 = p * 64 + h * 32 + 16 + e * 8
                    xea[t, r0:r0 + 8, ok] = x4[s, ch, pos[ok]]
    return xea.reshape(4 * 128, M).astype(NP_BF16)


def marshal_x(x4):
    """[4, 32, L] -> [16*128, J] bf16: u = g*4+s, row PERM[ph]*8+c, col j
    holds position 16*(j-1)+ph (zero-padded)."""
    xp = np.zeros((4, D, L + 32), np.float32)
    xp[:, :, 16:16 + L] = x4
    xph = xp.reshape(4, D, J, 16)[:, :, :, INVPERM]            # [s, ch, j, p]
    xr = xph.reshape(4, 4, 8, J, 16).transpose(1, 0, 4, 2, 3)  # [g, s, p, c, j]
    return np.ascontiguousarray(xr.reshape(16 * 128, J)).astype(NP_BF16)


def unmarshal_y(Y):
    """[16*64, M] bf16 -> [4, 16*L] fp32. Row = u*64 + r*4 + o, u = g*4+s."""
    t = np.asarray(Y, dtype=np.float32).reshape(16, 16, 4, M)   # [u, r, o, m]
    t = t.transpose(0, 2, 3, 1).reshape(16, 4, L)               # [u, o, n]
    t = t.reshape(4, 4, 4, L).transpose(1, 0, 2, 3)             # [s, g, o, n]
    return np.ascontiguousarray(t.reshape(4, 16 * L))


_program_cache = {}

# Set PROFILE=True (e.g. from a test harness) to capture an NTFF profile;
# the BassKernelResults lands in LAST_RESULT.
PROFILE = False
LAST_RESULT = None


def kernel(batch_x, kernels0, kernels1, kernels2, kernels3):
    global LAST_RESULT
    batch_x = np.asarray(batch_x)
    kernels = [np.asarray(k) for k in (kernels0, kernels1, kernels2, kernels3)]
    wm, wl, wr = build_weights(kernels)
    wea = build_wea(kernels)

    key = ("nc", K_AUX)
    if key not in _program_cache:
        _program_cache[key] = build_program(K_AUX)
    nc = _program_cache[key]

    x = batch_x.reshape(B, D, L)
    zero_xea = np.zeros((4 * 128, M), NP_BF16)
    in_maps = []
    for k in range(N_CORES):
        x4 = x[S * k:S * (k + 1)]
        in_maps.append({
            "x": marshal_x(x4),
            "xea": marshal_xea(x4) if K_AUX > 0 else zero_xea,
            "wm": wm, "wl": wl, "wr": wr, "wea": wea,
        })
    res = run_bass_kernel_spmd(nc, in_maps, list(range(N_CORES)), trace=PROFILE)
    LAST_RESULT = res
    ys = [unmarshal_y(res.results[k]["y"]) for k in range(N_CORES)]
    return np.concatenate(ys, axis=0)
